# revision 46
# baseline (speedup 1.0000x reference)
"""Causal multi-head attention on 8 Trainium2 NeuronCores (Bass/Tile).

Problem: B=4, N=2048, H=16, Hd=64 fp32 causal MHA.
Sharding: batch x head-group. Core c handles batch b=c//2 and heads
[8*(c%2), 8*(c%2)+8) -- 8 of 64 (b,h) slices, no cross-core communication.

v2 schedule (ScalarE exp is the bottleneck at ~153us busy; everything is
arranged to keep it fed):
  - Additive causal mask folded into the QK PSUM accumulation via an
    identity-matmul pre-write (start=True writes -1e9 blocks, QK accumulates
    on top with start=False). No per-block P multiply on GpSimd.
  - Global cross-head software pipeline: one pending-PV FIFO (PIPE deep)
    carried across heads and i-tiles, so PE interleaves head h+1's QK with
    head h's residual PVs and Act never sees a head-transition bubble.
  - Normalization split into tick-scheduled stages (PSUM->SBUF copy, PE
    re-transpose + reciprocal + scale, per-head-pair output DMA).
  - Input staging is need-ordered batched DMAs; the second half of K/Q/V
    streams in and is transposed between it=0 heads.
"""

from contextlib import ExitStack

import numpy as np

F32 = None  # set by _lazy_imports()
BF16 = None
HD = 64

B, N, H = 4, 2048, 16
N_CORES = 8
HEADS_PER_CORE = 8
D_CORE = HEADS_PER_CORE * HD

_cache = {}


def _lazy_imports():
    global F32, BF16, bacc, mybir, tile, bass_utils, make_identity, ml_dtypes
    import ml_dtypes as _mld

    import concourse.bacc as _bacc
    import concourse.mybir as _mybir
    import concourse.tile as _tile
    from concourse import bass_utils as _bu
    from concourse.masks import make_identity as _mi

    ml_dtypes = _mld
    bacc = _bacc
    mybir = _mybir
    tile = _tile
    bass_utils = _bu
    make_identity = _mi
    F32 = mybir.dt.float32
    BF16 = mybir.dt.bfloat16


def classify_mask(mask: np.ndarray):
    """Classify transposed 128x128 blocks of the attention mask.

    btype[(jb, ib)] in {'T', 'F', int index into blocks}. blocks are
    deduplicated ADDITIVE mixed blocks in S^T orientation (0 where allowed,
    -1e9 where masked); the last is the all-masked block used for F blocks
    that fall inside a conservative column cover."""
    S = mask.shape[0]
    nb = S // 128
    btype = {}
    blocks = []
    block_ids = {}
    for jb in range(nb):
        for ib in range(nb):
            blk = mask[ib * 128 : (ib + 1) * 128, jb * 128 : (jb + 1) * 128]
            if blk.all():
                btype[(jb, ib)] = "T"
            elif not blk.any():
                btype[(jb, ib)] = "F"
            else:
                key = blk.tobytes()
                if key not in block_ids:
                    block_ids[key] = len(blocks)
                    blocks.append(np.where(blk.T, 0.0, -1e9).astype(np.float32))
                btype[(jb, ib)] = block_ids[key]
    zero_idx = len(blocks)
    blocks.append(np.full((128, 128), -1e9, np.float32))
    return btype, blocks, zero_idx


def build_attn(
    n_cores,
    seq,
    heads,
    btype,
    n_blocks,
    zero_idx,
    i_tile=1024,
    phase_barrier=False,
    repeat=1,
    skip=frozenset(),
):
    D = heads * HD
    nb = seq // 128
    n_it = seq // i_tile
    n_dt = (D + 127) // 128
    n_ch = i_tile // 512
    scale = 1.0 / np.sqrt(HD)
    PIPE = 8

    nc = bacc.Bacc("TRN2", target_bir_lowering=False, debug=False, num_devices=n_cores)
    qs = nc.dram_tensor("qs", [seq, D], F32, kind="ExternalInput").ap()
    ks = nc.dram_tensor("ks", [seq, D], F32, kind="ExternalInput").ap()
    vs = nc.dram_tensor("vs", [seq, D], F32, kind="ExternalInput").ap()
    mblk = nc.dram_tensor("mblk", [n_blocks, 128, 128], BF16, kind="ExternalInput").ap()
    ys = nc.dram_tensor("ys", [seq, D], F32, kind="ExternalOutput").ap()

    with tile.TileContext(nc) as tc, ExitStack() as ctx:
        singles = ctx.enter_context(tc.tile_pool(name="singles", bufs=1))
        natp = ctx.enter_context(tc.tile_pool(name="natp", bufs=4))
        ptp = ctx.enter_context(tc.tile_pool(name="ptp", bufs=PIPE + 2))
        outp = ctx.enter_context(tc.tile_pool(name="outp", bufs=3))
        stgp = ctx.enter_context(tc.tile_pool(name="stgp", bufs=3))
        recp = ctx.enter_context(tc.tile_pool(name="recp", bufs=4))
        stp = ctx.enter_context(tc.tile_pool(name="stp", bufs=2, space="PSUM"))
        pvp = ctx.enter_context(tc.tile_pool(name="pvp", bufs=1, space="PSUM"))
        tpp = ctx.enter_context(tc.tile_pool(name="tpp", bufs=2, space="PSUM"))

        def body():
            # Warm-up ACTIVATE: forces the exp table-set load during staging,
            # long before the real exps.
            warm = singles.tile([1, 8], F32, name="warm")
            nc.vector.memset(warm, 0.0)
            nc.scalar.activation(
                out=warm, in_=warm, func=mybir.ActivationFunctionType.Exp
            )

            ident = singles.tile([128, 128], F32, name="ident")
            make_identity(nc, ident)
            identb = singles.tile([128, 128], BF16, name="identb")
            make_identity(nc, identb)
            msb = singles.tile([128, n_blocks * 128], BF16, name="msb")
            for m in range(n_blocks):
                nc.sync.dma_start(out=msb[:, m * 128 : (m + 1) * 128], in_=mblk[m])

            # per-128-col-block transposed tiles: separate tiles keep the
            # byte-range dependency tracker exact (one big [128,n_dt,seq]
            # tile creates bounding-box false deps serializing QK behind ALL
            # staging writes)
            qTt = {}
            kTt = {}
            vpt = {}

            def load(src, t0, nt, tag):
                nat = natp.tile(
                    [128, nt, D],
                    F32,
                    tag=tag,
                    bufs={"natL": 6, "nat2": 6, "nat4": 4}[tag],
                    name=f"nat_{t0}_{nt}",
                )
                nc.sync.dma_start(
                    out=nat,
                    in_=src[t0 * 128 : (t0 + nt) * 128, :].rearrange(
                        "(a p) e -> p a e", p=128
                    ),
                )
                return nat

            def stage_qk(tiles, nat, a, t):
                # Pool casts fp32->bf16, then bf16 PE transpose (1 cyc/col
                # vs 2 for fp32) into PSUM, VectorE copy out (2x bf16 mode)
                natb = natp.tile([128, D], BF16, tag="natb", bufs=3, name="natb")
                nc.gpsimd.tensor_copy(natb, nat[:, a, :])
                tpq = tpp.tile([128, n_dt, 128], BF16, tag="tp")
                for td in range(n_dt):
                    nc.tensor.transpose(
                        tpq[:, td, :], natb[:, td * 128 : (td + 1) * 128], identb
                    )
                tt = singles.tile(
                    [128, n_dt, 128], BF16, name=f"{'q' if tiles is qTt else 'k'}T{t}"
                )
                nc.vector.tensor_copy(tt, tpq)
                tiles[t] = tt

            def stage_v(nat, a, t):
                vp = singles.tile([128, heads, HD + 1], BF16, name=f"vp{t}")
                nc.gpsimd.tensor_copy(
                    vp[:, :, 0:HD], nat[:, a, :].rearrange("p (h e) -> p h e", h=heads)
                )
                nc.gpsimd.memset(vp[:, :, HD : HD + 1], 1.0)
                vpt[t] = vp

            # ---- early staging: first half of K/Q/V ----
            # Emission order = need order: the tpq PSUM ring (bufs=2)
            # strictly serializes transpose groups, so a group emitted early
            # but whose DMA lands late would block every later group.
            half = nb // 2
            k02 = load(ks, 0, 2, "nat2")
            q02 = load(qs, 0, 2, "nat2")
            q24 = load(qs, 2, 2, "nat2")
            k24 = load(ks, 2, 2, "nat2")
            q48 = load(qs, 4, 4, "nat4")
            v02 = load(vs, 0, 2, "nat2")
            k48 = load(ks, 4, 4, "nat4")
            v24 = load(vs, 2, 2, "nat2")
            v48 = load(vs, 4, 4, "nat4")
            for t in range(2):
                stage_qk(kTt, k02, t, t)
            for t in range(2):
                stage_qk(qTt, q02, t, t)
            for t in range(2):
                stage_qk(qTt, q24, t, 2 + t)
            for t in range(2):
                stage_qk(kTt, k24, t, 2 + t)
            for t in range(4):
                stage_qk(qTt, q48, t, 4 + t)
            for t in range(2):
                stage_v(v02, t, t)
            for t in range(4):
                stage_qk(kTt, k48, t, 4 + t)
            for t in range(2):
                stage_v(v24, t, 2 + t)
            for t in range(4):
                stage_v(v48, t, 4 + t)
            if n_it == 1:
                # no second i-tile to hide late staging behind: stage it all now
                for t0 in (half, half + half // 2):
                    natk = load(ks, t0, half // 2, "natL")
                    natq = load(qs, t0, half // 2, "natL")
                    natv = load(vs, t0, half // 2, "natL")
                    for t in range(half // 2):
                        stage_qk(kTt, natk, t, t0 + t)
                        stage_qk(qTt, natq, t, t0 + t)
                        stage_v(natv, t, t0 + t)

            late = {}  # name -> nat tile

            # ---- main cross-head pipelined loop ----
            pending = []  # (pt, pv, h, jbs_entry, cf, cl, i0, last_of_slot)
            norm_q = []  # (due_tick, stage, payload)
            filler_q = []  # late staging closures, paced between groups
            tick = [0]
            stg_tiles = {}

            def emit_stage1(payload):
                it, h, pv = payload
                outT = outp.tile([HD + 1, i_tile], F32, tag="outT", name="outT")
                # split copy so PV(next slot) WAR-waits only on the half it
                # touches first (subtile deps), not the full 1024-col copy
                nc.vector.tensor_copy(outT[:, 0:512], pv[:, 0:512])
                nc.vector.tensor_copy(outT[:, 512:i_tile], pv[:, 512:i_tile])
                norm_q.append((tick[0] + 3, 2, (it, h, outT)))
                norm_q.sort(key=lambda e: e[0])

            def emit_stage2(payload):
                it, h, outT = payload
                i0 = it * i_tile
                pair = h // 2
                final = it == n_it - 1 and h == heads - 1
                if h % 2 == 0:
                    stg = stgp.tile(
                        [128, i_tile // 128, 2, HD], F32, tag="stg", name="stg"
                    )
                    stg_tiles[(it, pair)] = stg
                else:
                    stg = stg_tiles[(it, pair)]
                for g in range(i_tile // 512):
                    tp = tpp.tile([128, 4, HD + 2], F32, tag="tp", name="tpo")
                    for c4 in range(4):
                        ch = g * 4 + c4
                        nc.tensor.transpose(
                            tp[:, c4, 0 : HD + 1],
                            outT[:, ch * 128 : (ch + 1) * 128],
                            ident[0 : HD + 1, 0 : HD + 1],
                        )
                    rec = recp.tile([128, 4], F32, tag="rec", name="rec")
                    nc.vector.reciprocal(rec, tp[:, :, HD])
                    for c4 in range(4):
                        ch = g * 4 + c4
                        nc.vector.tensor_scalar_mul(
                            stg[:, ch, h % 2, :], tp[:, c4, 0:HD], rec[:, c4 : c4 + 1]
                        )
                if h % 2 == 1:
                    nc.sync.dma_start(
                        out=ys[i0 : i0 + i_tile, pair * 128 : (pair + 1) * 128].rearrange(
                            "(c p) e -> p c e", p=128
                        ),
                        in_=stg.rearrange("p c h e -> p c (h e)"),
                    )

            def emit_stage3(payload):
                # eager per-512-group norm + output DMA for the final slot
                it, h, outT, g = payload
                i0 = it * i_tile
                pair = h // 2
                stg = stg_tiles[(it, pair)]
                tp = tpp.tile([128, 4, HD + 2], F32, tag="tp", name="tpo")
                for c4 in range(4):
                    ch = g * 4 + c4
                    nc.tensor.transpose(
                        tp[:, c4, 0 : HD + 1],
                        outT[:, ch * 128 : (ch + 1) * 128],
                        ident[0 : HD + 1, 0 : HD + 1],
                    )
                rec = recp.tile([128, 4], F32, tag="rec", name="rec")
                nc.vector.reciprocal(rec, tp[:, :, HD])
                for c4 in range(4):
                    ch = g * 4 + c4
                    if g == n_ch - 1 and c4 % 2 == 1:
                        # tail: ScalarE is idle, split the muls across engines
                        nc.scalar.activation(
                            out=stg[:, ch, h % 2, :],
                            in_=tp[:, c4, 0:HD],
                            func=mybir.ActivationFunctionType.Copy,
                            scale=rec[:, c4 : c4 + 1],
                        )
                    else:
                        nc.vector.tensor_scalar_mul(
                            stg[:, ch, h % 2, :], tp[:, c4, 0:HD], rec[:, c4 : c4 + 1]
                        )
                nc.sync.dma_start(
                    out=ys[
                        i0 + g * 512 : i0 + (g + 1) * 512,
                        pair * 128 : (pair + 1) * 128,
                    ].rearrange("(c p) e -> p c e", p=128),
                    in_=stg[:, g * 4 : (g + 1) * 4, :, :].rearrange(
                        "p c h e -> p c (h e)"
                    ),
                )

            def service():
                while norm_q and norm_q[0][0] <= tick[0]:
                    _, stage, payload = norm_q.pop(0)
                    if stage == 1:
                        emit_stage1(payload)
                    elif stage == 2:
                        emit_stage2(payload)
                    else:
                        emit_stage3(payload)

            eager_state = {}

            def run_pv(ent):
                pt, pv, h, (jb, lo, hi, po), cf, cl, i0, last = ent
                closed = []
                for c in range(n_ch):
                    a, b = max(lo, c * 512), min(hi, (c + 1) * 512)
                    if a >= b:
                        continue
                    nc.tensor.matmul(
                        pv[:, a:b],
                        lhsT=vpt[jb][:, h, :],
                        rhs=pt[:, po + a - lo : po + b - lo],
                        start=(jb == cf[c]),
                        stop=(jb == cl[c]),
                    )
                    if jb == cl[c]:
                        closed.append(c)
                it = i0 // i_tile
                if (i0, h) in eager_state and closed:
                    # final slot: copy each pv chunk out as its accumulation
                    # closes and norm + DMA that group immediately, so the
                    # tail after the last exp is one short group, not a full
                    # head normalization
                    es = eager_state[(i0, h)]
                    if es["outT"] is None:
                        es["outT"] = outp.tile(
                            [HD + 1, i_tile], F32, tag="outT", name="outT"
                        )
                    for c in closed:
                        if c == n_ch - 1:
                            # last chunk closes after the final exp: ScalarE
                            # is idle, keep DVE off the tail critical path
                            nc.scalar.activation(
                                out=es["outT"][:, c * 512 : (c + 1) * 512],
                                in_=pv[:, c * 512 : (c + 1) * 512],
                                func=mybir.ActivationFunctionType.Copy,
                            )
                        else:
                            nc.vector.tensor_copy(
                                es["outT"][:, c * 512 : (c + 1) * 512],
                                pv[:, c * 512 : (c + 1) * 512],
                            )
                        norm_q.append((tick[0], 3, (it, h, es["outT"], c)))
                    norm_q.sort(key=lambda e: e[0])
                elif last is not None:
                    it, h = last
                    norm_q.append((tick[0] + 1, 1, (it, h, pv)))
                    norm_q.sort(key=lambda e: e[0])

            def emit_slot(
                it, h, pipe_depth, ilo=0, ihi=None, pv=None, close=True, reorder=True
            ):
                i0 = it * i_tile
                if ihi is None:
                    ihi = i_tile
                jbs = []
                for jb in range(nb):
                    ics = [
                        ic
                        for ic in range((i0 + ilo) // 128, (i0 + ihi) // 128)
                        if btype[(jb, ic)] != "F"
                    ]
                    if ics:
                        jbs.append(
                            [jb, min(ics) * 128 - i0, max(ics) * 128 + 128 - i0]
                        )
                if reorder and len(jbs) > 2:
                    # pack complementary-length jbs into shared st tiles /
                    # exp instructions (fewer ScalarE per-instr overheads),
                    # and end the slot on LONG exps: the boundary st-ring
                    # WAR + next head's first QK must hide under them
                    rest = sorted(jbs[1:], key=lambda e: e[2] - e[1])
                    groups = []
                    i, j = 0, len(rest) - 1
                    while i <= j:
                        if i < j and (rest[i][2] - rest[i][1]) + (
                            rest[j][2] - rest[j][1]
                        ) <= i_tile:
                            groups.append([rest[i], rest[j]])
                            i += 1
                            j -= 1
                        else:
                            groups.append([rest[j]])
                            j -= 1
                    groups.sort(key=lambda g: sum(e[2] - e[1] for e in g))
                    groups = [[jbs[0]]] + groups
                else:
                    groups = [[e] for e in jbs]
                jbs = [e for g in groups for e in g]
                cf = {}
                cl = {}
                for jb, lo, hi in jbs:
                    for c in range(ilo // 512, (ihi + 511) // 512):
                        if lo < (c + 1) * 512 and hi > c * 512:
                            if c not in cf:
                                cf[c] = jb
                            cl[c] = jb
                for ent in jbs:
                    for c in range(ilo // 512, (ihi + 511) // 512):
                        if cf.get(c) == ent[0]:
                            ent[1] = min(ent[1], max(c * 512, ilo))
                            ent[2] = max(ent[2], min((c + 1) * 512, ihi))

                if pv is None:
                    pv = pvp.tile([HD + 1, i_tile], F32, tag="pv", name="pv")
                eff_depth = min(pipe_depth, max(3, len(jbs) // 2))
                h2, poff = (h * HD) // 128, (h * HD) % 128
                for gdx, group in enumerate(groups):
                    st = stp.tile([128, i_tile], F32, tag="st", name="st")
                    off = 0
                    members = []
                    for jb, lo, hi in group:
                        kslice = kTt[jb][poff : poff + HD, h2, :]
                        for l in range(lo, hi, 128):
                            p = off + (l - lo)
                            ic = (i0 + l) // 128
                            bt = btype[(jb, ic)]
                            masked = bt != "T"
                            if masked:
                                # additive mask pre-write; QK accumulates on
                                # top with start=False
                                bi = zero_idx if bt == "F" else bt
                                nc.tensor.matmul(
                                    st[:, p : p + 128],
                                    lhsT=identb,
                                    rhs=msb[:, bi * 128 : (bi + 1) * 128],
                                    start=True,
                                    stop=False,
                                )
                            nc.tensor.matmul(
                                st[:, p : p + 128],
                                lhsT=kslice,
                                rhs=qTt[ic][poff : poff + HD, h2, :],
                                start=not masked,
                                stop=True,
                            )
                        members.append((jb, lo, hi, off))
                        off += hi - lo
                    service()
                    pops = 0
                    while len(pending) >= eff_depth and pops < 2:
                        run_pv(pending.pop(0))
                        pops += 1
                    if filler_q and tick[0] % 2 == 0:
                        filler_q.pop(0)()
                    pt = ptp.tile([128, i_tile], BF16, tag="pt", name="pt")
                    nc.scalar.activation(
                        out=pt[:, 0:off],
                        in_=st[:, 0:off],
                        func=mybir.ActivationFunctionType.Exp,
                        scale=float(scale),
                    )
                    for mdx, (jb, lo, hi, po) in enumerate(members):
                        last = (
                            (it, h)
                            if close
                            and gdx == len(groups) - 1
                            and mdx == len(members) - 1
                            else None
                        )
                        pending.append((pt, pv, h, (jb, lo, hi, po), cf, cl, i0, last))
                    tick[0] += 1
                return pv

            # ---- slot schedule with late-staging injections ----
            n_slots = n_it * heads
            for si in range(n_slots):
                it, h = si // heads, si % heads
                if it > 0:
                    # all late staging must be emitted before i-tile 1 reads it
                    while filler_q:
                        filler_q.pop(0)()
                depth = PIPE
                if si == n_slots - 1:
                    depth = 3
                    eager_state[((n_it - 1) * i_tile, heads - 1)] = {"outT": None}
                if si == 0 and i_tile > 512:
                    # sub-range the first slot so the first exps only need
                    # the first Q/K eighth (staging DMA latency); forward jb
                    # order matches staging arrival order
                    pv0 = emit_slot(it, h, depth, 0, 256, close=False, reorder=False)
                    emit_slot(it, h, depth, 256, 512, pv=pv0, close=False, reorder=False)
                    emit_slot(it, h, depth, 512, i_tile, pv=pv0, close=True, reorder=False)
                else:
                    # final slot: forward order so chunk 0 closes early for
                    # the eager tail norm
                    emit_slot(it, h, depth, reorder=(si != n_slots - 1))
                if it == 0 and n_it > 1:
                    if h == 0:
                        late["k8"] = load(ks, half, half // 2, "natL")
                        late["k12"] = load(ks, half + half // 2, half // 2, "natL")

                        def mk_qk(tiles, nat, a, t):
                            return lambda: stage_qk(tiles, nat, a, t)

                        for t in range(half // 2):
                            filler_q.append(mk_qk(kTt, late["k8"], t, half + t))
                    elif h == 1:
                        late["q8"] = load(qs, half, half // 2, "natL")
                        late["q12"] = load(qs, half + half // 2, half // 2, "natL")

                        def mk_qk(tiles, nat, a, t):
                            return lambda: stage_qk(tiles, nat, a, t)

                        for t in range(half // 2):
                            filler_q.append(
                                mk_qk(kTt, late["k12"], t, half + half // 2 + t)
                            )
                        for t in range(half // 2):
                            filler_q.append(mk_qk(qTt, late["q8"], t, half + t))
                        for t in range(half // 2):
                            filler_q.append(
                                mk_qk(qTt, late["q12"], t, half + half // 2 + t)
                            )
                    elif h == 2:
                        late["v8"] = load(vs, half, half // 2, "natL")
                        late["v12"] = load(vs, half + half // 2, half // 2, "natL")
                    elif h == 6:
                        for t in range(half // 2):
                            stage_v(late["v8"], t, half + t)
                    elif h == 7:
                        for t in range(half // 2):
                            stage_v(late["v12"], t, half + half // 2 + t)

            # ---- drain ----
            while pending:
                run_pv(pending.pop(0))
                tick[0] += 1
                service()
            tick[0] += 10**6
            service()

        if repeat == 1:
            body()
        else:
            with tc.For_i(0, repeat, 1):
                body()

    nc.compile()
    return nc


def _get_program(mask: np.ndarray):
    _lazy_imports()
    key = hash(mask.tobytes())
    if key not in _cache:
        btype, blocks, zero_idx = classify_mask(mask)
        mblk = np.stack(blocks).astype(ml_dtypes.bfloat16)
        nc = build_attn(
            n_cores=N_CORES,
            seq=N,
            heads=HEADS_PER_CORE,
            btype=btype,
            n_blocks=len(blocks),
            zero_idx=zero_idx,
            i_tile=1024,
        )
        _cache[key] = (nc, mblk)
    return _cache[key]


def make_in_maps(q, k, v, mblk):
    in_maps = []
    for c in range(N_CORES):
        b, dg = c // 2, D_CORE * (c % 2)
        in_maps.append(
            {
                "qs": np.ascontiguousarray(q[b][:, dg : dg + D_CORE]),
                "ks": np.ascontiguousarray(k[b][:, dg : dg + D_CORE]),
                "vs": np.ascontiguousarray(v[b][:, dg : dg + D_CORE]),
                "mblk": mblk,
            }
        )
    return in_maps


def gather_out(results):
    y = np.empty((B, N, H * HD), np.float32)
    for c in range(N_CORES):
        b, dg = c // 2, D_CORE * (c % 2)
        y[b][:, dg : dg + D_CORE] = results[c]["ys"]
    return y


def kernel(q, k, v, attn_mask):
    q = np.asarray(q, np.float32)
    k = np.asarray(k, np.float32)
    v = np.asarray(v, np.float32)
    mask = np.asarray(attn_mask, bool)
    nc, mblk = _get_program(mask)
    res = bass_utils.run_bass_kernel_spmd(
        nc, make_in_maps(q, k, v, mblk), core_ids=list(range(N_CORES))
    )
    return gather_out(res.results)


# revision 47
# speedup vs baseline: 1.0158x; 1.0158x over previous
"""Causal multi-head attention on 8 Trainium2 NeuronCores (Bass/Tile).

Problem: B=4, N=2048, H=16, Hd=64 fp32 causal MHA.
Sharding: batch x head-group. Core c handles batch b=c//2 and heads
[8*(c%2), 8*(c%2)+8) -- 8 of 64 (b,h) slices, no cross-core communication.

v2 schedule (ScalarE exp is the bottleneck at ~153us busy; everything is
arranged to keep it fed):
  - Additive causal mask folded into the QK PSUM accumulation via an
    identity-matmul pre-write (start=True writes -1e9 blocks, QK accumulates
    on top with start=False). No per-block P multiply on GpSimd.
  - Global cross-head software pipeline: one pending-PV FIFO (PIPE deep)
    carried across heads and i-tiles, so PE interleaves head h+1's QK with
    head h's residual PVs and Act never sees a head-transition bubble.
  - Normalization split into tick-scheduled stages (PSUM->SBUF copy, PE
    re-transpose + reciprocal + scale, per-head-pair output DMA).
  - Input staging is need-ordered batched DMAs; the second half of K/Q/V
    streams in and is transposed between it=0 heads.
"""

from contextlib import ExitStack

import numpy as np

F32 = None  # set by _lazy_imports()
BF16 = None
HD = 64

B, N, H = 4, 2048, 16
N_CORES = 8
HEADS_PER_CORE = 8
D_CORE = HEADS_PER_CORE * HD

_cache = {}


def _lazy_imports():
    global F32, BF16, bacc, mybir, tile, bass_utils, make_identity, ml_dtypes
    import ml_dtypes as _mld

    import concourse.bacc as _bacc
    import concourse.mybir as _mybir
    import concourse.tile as _tile
    from concourse import bass_utils as _bu
    from concourse.masks import make_identity as _mi

    ml_dtypes = _mld
    bacc = _bacc
    mybir = _mybir
    tile = _tile
    bass_utils = _bu
    make_identity = _mi
    F32 = mybir.dt.float32
    BF16 = mybir.dt.bfloat16


def classify_mask(mask: np.ndarray):
    """Classify transposed 128x128 blocks of the attention mask.

    btype[(jb, ib)] in {'T', 'F', int index into blocks}. blocks are
    deduplicated ADDITIVE mixed blocks in S^T orientation (0 where allowed,
    -1e9 where masked); the last is the all-masked block used for F blocks
    that fall inside a conservative column cover."""
    S = mask.shape[0]
    nb = S // 128
    btype = {}
    blocks = []
    block_ids = {}
    for jb in range(nb):
        for ib in range(nb):
            blk = mask[ib * 128 : (ib + 1) * 128, jb * 128 : (jb + 1) * 128]
            if blk.all():
                btype[(jb, ib)] = "T"
            elif not blk.any():
                btype[(jb, ib)] = "F"
            else:
                key = blk.tobytes()
                if key not in block_ids:
                    block_ids[key] = len(blocks)
                    blocks.append(np.where(blk.T, 0.0, -1e9).astype(np.float32))
                btype[(jb, ib)] = block_ids[key]
    zero_idx = len(blocks)
    blocks.append(np.full((128, 128), -1e9, np.float32))
    return btype, blocks, zero_idx


def build_attn(
    n_cores,
    seq,
    heads,
    btype,
    n_blocks,
    zero_idx,
    i_tile=1024,
    phase_barrier=False,
    repeat=1,
    skip=frozenset(),
):
    D = heads * HD
    nb = seq // 128
    n_it = seq // i_tile
    n_dt = (D + 127) // 128
    n_ch = i_tile // 512
    scale = 1.0 / np.sqrt(HD)
    PIPE = 8

    nc = bacc.Bacc("TRN2", target_bir_lowering=False, debug=False, num_devices=n_cores)
    qs = nc.dram_tensor("qs", [seq, D], F32, kind="ExternalInput").ap()
    ks = nc.dram_tensor("ks", [seq, D], F32, kind="ExternalInput").ap()
    vs = nc.dram_tensor("vs", [seq, D], F32, kind="ExternalInput").ap()
    mblk = nc.dram_tensor("mblk", [n_blocks, 128, 128], BF16, kind="ExternalInput").ap()
    ys = nc.dram_tensor("ys", [seq, D], F32, kind="ExternalOutput").ap()

    with tile.TileContext(nc) as tc, ExitStack() as ctx:
        singles = ctx.enter_context(tc.tile_pool(name="singles", bufs=1))
        natp = ctx.enter_context(tc.tile_pool(name="natp", bufs=4))
        ptp = ctx.enter_context(tc.tile_pool(name="ptp", bufs=PIPE + 2))
        outp = ctx.enter_context(tc.tile_pool(name="outp", bufs=3))
        stgp = ctx.enter_context(tc.tile_pool(name="stgp", bufs=3))
        recp = ctx.enter_context(tc.tile_pool(name="recp", bufs=4))
        stp = ctx.enter_context(tc.tile_pool(name="stp", bufs=2, space="PSUM"))
        pvp = ctx.enter_context(tc.tile_pool(name="pvp", bufs=1, space="PSUM"))
        tpp = ctx.enter_context(tc.tile_pool(name="tpp", bufs=2, space="PSUM"))

        def body():
            # Warm-up ACTIVATE: forces the exp table-set load during staging,
            # long before the real exps.
            warm = singles.tile([1, 8], F32, name="warm")
            nc.vector.memset(warm, 0.0)
            nc.scalar.activation(
                out=warm, in_=warm, func=mybir.ActivationFunctionType.Exp
            )

            ident = singles.tile([128, 128], F32, name="ident")
            make_identity(nc, ident)
            identb = singles.tile([128, 128], BF16, name="identb")
            make_identity(nc, identb)
            msb = singles.tile([128, n_blocks * 128], BF16, name="msb")
            for m in range(n_blocks):
                nc.sync.dma_start(out=msb[:, m * 128 : (m + 1) * 128], in_=mblk[m])

            # per-128-col-block transposed tiles: separate tiles keep the
            # byte-range dependency tracker exact (one big [128,n_dt,seq]
            # tile creates bounding-box false deps serializing QK behind ALL
            # staging writes)
            qTt = {}
            kTt = {}
            vpt = {}

            def load(src, t0, nt, tag):
                nat = natp.tile(
                    [128, nt, D],
                    F32,
                    tag=tag,
                    bufs={"natL": 6, "nat2": 6, "nat4": 4}[tag],
                    name=f"nat_{t0}_{nt}",
                )
                nc.sync.dma_start(
                    out=nat,
                    in_=src[t0 * 128 : (t0 + nt) * 128, :].rearrange(
                        "(a p) e -> p a e", p=128
                    ),
                )
                return nat

            def stage_qk(tiles, nat, a, t):
                # Pool casts fp32->bf16, then bf16 PE transpose (1 cyc/col
                # vs 2 for fp32) into PSUM, VectorE copy out (2x bf16 mode)
                natb = natp.tile([128, D], BF16, tag="natb", bufs=3, name="natb")
                nc.gpsimd.tensor_copy(natb, nat[:, a, :])
                tpq = tpp.tile([128, n_dt, 128], BF16, tag="tp")
                for td in range(n_dt):
                    nc.tensor.transpose(
                        tpq[:, td, :], natb[:, td * 128 : (td + 1) * 128], identb
                    )
                tt = singles.tile(
                    [128, n_dt, 128], BF16, name=f"{'q' if tiles is qTt else 'k'}T{t}"
                )
                nc.vector.tensor_copy(tt, tpq)
                tiles[t] = tt

            def stage_v(nat, a, t):
                vp = singles.tile([128, heads, HD + 1], BF16, name=f"vp{t}")
                nc.gpsimd.tensor_copy(
                    vp[:, :, 0:HD], nat[:, a, :].rearrange("p (h e) -> p h e", h=heads)
                )
                nc.gpsimd.memset(vp[:, :, HD : HD + 1], 1.0)
                vpt[t] = vp

            # ---- early staging: first half of K/Q/V ----
            # Emission order = need order: the tpq PSUM ring (bufs=2)
            # strictly serializes transpose groups, so a group emitted early
            # but whose DMA lands late would block every later group.
            half = nb // 2
            k02 = load(ks, 0, 2, "nat2")
            q02 = load(qs, 0, 2, "nat2")
            q24 = load(qs, 2, 2, "nat2")
            k24 = load(ks, 2, 2, "nat2")
            q48 = load(qs, 4, 4, "nat4")
            v02 = load(vs, 0, 2, "nat2")
            k48 = load(ks, 4, 4, "nat4")
            v24 = load(vs, 2, 2, "nat2")
            v48 = load(vs, 4, 4, "nat4")
            for t in range(2):
                stage_qk(kTt, k02, t, t)
            for t in range(2):
                stage_qk(qTt, q02, t, t)
            for t in range(2):
                stage_qk(qTt, q24, t, 2 + t)
            for t in range(2):
                stage_qk(kTt, k24, t, 2 + t)
            for t in range(4):
                stage_qk(qTt, q48, t, 4 + t)
            for t in range(2):
                stage_v(v02, t, t)
            for t in range(4):
                stage_qk(kTt, k48, t, 4 + t)
            for t in range(2):
                stage_v(v24, t, 2 + t)
            for t in range(4):
                stage_v(v48, t, 4 + t)
            if n_it == 1:
                # no second i-tile to hide late staging behind: stage it all now
                for t0 in (half, half + half // 2):
                    natk = load(ks, t0, half // 2, "natL")
                    natq = load(qs, t0, half // 2, "natL")
                    natv = load(vs, t0, half // 2, "natL")
                    for t in range(half // 2):
                        stage_qk(kTt, natk, t, t0 + t)
                        stage_qk(qTt, natq, t, t0 + t)
                        stage_v(natv, t, t0 + t)

            late = {}  # name -> nat tile

            # ---- main cross-head pipelined loop ----
            pending = []  # (pt, pv, h, jbs_entry, cf, cl, i0, last_of_slot)
            norm_q = []  # (due_tick, stage, payload)
            filler_q = []  # late staging closures, paced between groups
            tick = [0]
            stg_tiles = {}

            def emit_stage1(payload):
                it, h, pv = payload
                outT = outp.tile([HD + 1, i_tile], F32, tag="outT", name="outT")
                # split copy so PV(next slot) WAR-waits only on the half it
                # touches first (subtile deps), not the full 1024-col copy
                nc.vector.tensor_copy(outT[:, 0:512], pv[:, 0:512])
                nc.vector.tensor_copy(outT[:, 512:i_tile], pv[:, 512:i_tile])
                norm_q.append((tick[0] + 3, 2, (it, h, outT)))
                norm_q.sort(key=lambda e: e[0])

            def emit_stage2(payload):
                it, h, outT = payload
                i0 = it * i_tile
                pair = h // 2
                final = it == n_it - 1 and h == heads - 1
                if h % 2 == 0:
                    stg = stgp.tile(
                        [128, i_tile // 128, 2, HD], F32, tag="stg", name="stg"
                    )
                    stg_tiles[(it, pair)] = stg
                else:
                    stg = stg_tiles[(it, pair)]
                for g in range(i_tile // 512):
                    tp = tpp.tile([128, 4, HD + 2], F32, tag="tp", name="tpo")
                    for c4 in range(4):
                        ch = g * 4 + c4
                        nc.tensor.transpose(
                            tp[:, c4, 0 : HD + 1],
                            outT[:, ch * 128 : (ch + 1) * 128],
                            ident[0 : HD + 1, 0 : HD + 1],
                        )
                    rec = recp.tile([128, 4], F32, tag="rec", name="rec")
                    nc.vector.reciprocal(rec, tp[:, :, HD])
                    nc.vector.tensor_mul(
                        stg[:, g * 4 : (g + 1) * 4, h % 2, :],
                        tp[:, :, 0:HD],
                        rec.to_broadcast([128, 4, HD]),
                    )
                if h % 2 == 1:
                    nc.sync.dma_start(
                        out=ys[i0 : i0 + i_tile, pair * 128 : (pair + 1) * 128].rearrange(
                            "(c p) e -> p c e", p=128
                        ),
                        in_=stg.rearrange("p c h e -> p c (h e)"),
                    )

            def emit_stage3(payload):
                # eager per-512-group norm + output DMA for the final slot
                it, h, outT, g = payload
                i0 = it * i_tile
                pair = h // 2
                stg = stg_tiles[(it, pair)]
                tp = tpp.tile([128, 4, HD + 2], F32, tag="tp", name="tpo")
                for c4 in range(4):
                    ch = g * 4 + c4
                    nc.tensor.transpose(
                        tp[:, c4, 0 : HD + 1],
                        outT[:, ch * 128 : (ch + 1) * 128],
                        ident[0 : HD + 1, 0 : HD + 1],
                    )
                rec = recp.tile([128, 4], F32, tag="rec", name="rec")
                nc.vector.reciprocal(rec, tp[:, :, HD])
                nc.vector.tensor_mul(
                    stg[:, g * 4 : (g + 1) * 4, h % 2, :],
                    tp[:, :, 0:HD],
                    rec.to_broadcast([128, 4, HD]),
                )
                nc.sync.dma_start(
                    out=ys[
                        i0 + g * 512 : i0 + (g + 1) * 512,
                        pair * 128 : (pair + 1) * 128,
                    ].rearrange("(c p) e -> p c e", p=128),
                    in_=stg[:, g * 4 : (g + 1) * 4, :, :].rearrange(
                        "p c h e -> p c (h e)"
                    ),
                )

            def service():
                while norm_q and norm_q[0][0] <= tick[0]:
                    _, stage, payload = norm_q.pop(0)
                    if stage == 1:
                        emit_stage1(payload)
                    elif stage == 2:
                        emit_stage2(payload)
                    else:
                        emit_stage3(payload)

            eager_state = {}

            def run_pv(ent):
                pt, pv, h, (jb, lo, hi, po), cf, cl, i0, last = ent
                closed = []
                for c in range(n_ch):
                    a, b = max(lo, c * 512), min(hi, (c + 1) * 512)
                    if a >= b:
                        continue
                    nc.tensor.matmul(
                        pv[:, a:b],
                        lhsT=vpt[jb][:, h, :],
                        rhs=pt[:, po + a - lo : po + b - lo],
                        start=(jb == cf[c]),
                        stop=(jb == cl[c]),
                    )
                    if jb == cl[c]:
                        closed.append(c)
                it = i0 // i_tile
                if (i0, h) in eager_state and closed:
                    # final slot: copy each pv chunk out as its accumulation
                    # closes and norm + DMA that group immediately, so the
                    # tail after the last exp is one short group, not a full
                    # head normalization
                    es = eager_state[(i0, h)]
                    if es["outT"] is None:
                        es["outT"] = outp.tile(
                            [HD + 1, i_tile], F32, tag="outT", name="outT"
                        )
                    for c in closed:
                        if c == n_ch - 1:
                            # last chunk closes after the final exp: ScalarE
                            # is idle, keep DVE off the tail critical path
                            nc.scalar.activation(
                                out=es["outT"][:, c * 512 : (c + 1) * 512],
                                in_=pv[:, c * 512 : (c + 1) * 512],
                                func=mybir.ActivationFunctionType.Copy,
                            )
                        else:
                            nc.vector.tensor_copy(
                                es["outT"][:, c * 512 : (c + 1) * 512],
                                pv[:, c * 512 : (c + 1) * 512],
                            )
                        norm_q.append((tick[0], 3, (it, h, es["outT"], c)))
                    norm_q.sort(key=lambda e: e[0])
                elif last is not None:
                    it, h = last
                    norm_q.append((tick[0] + 1, 1, (it, h, pv)))
                    norm_q.sort(key=lambda e: e[0])

            def emit_slot(
                it, h, pipe_depth, ilo=0, ihi=None, pv=None, close=True, reorder=True
            ):
                i0 = it * i_tile
                if ihi is None:
                    ihi = i_tile
                jbs = []
                for jb in range(nb):
                    ics = [
                        ic
                        for ic in range((i0 + ilo) // 128, (i0 + ihi) // 128)
                        if btype[(jb, ic)] != "F"
                    ]
                    if ics:
                        jbs.append(
                            [jb, min(ics) * 128 - i0, max(ics) * 128 + 128 - i0]
                        )
                if reorder and len(jbs) > 2:
                    # pack complementary-length jbs into shared st tiles /
                    # exp instructions (fewer ScalarE per-instr overheads),
                    # and end the slot on LONG exps: the boundary st-ring
                    # WAR + next head's first QK must hide under them
                    rest = sorted(jbs[1:], key=lambda e: e[2] - e[1])
                    groups = []
                    i, j = 0, len(rest) - 1
                    while i <= j:
                        if i < j and (rest[i][2] - rest[i][1]) + (
                            rest[j][2] - rest[j][1]
                        ) <= i_tile:
                            groups.append([rest[i], rest[j]])
                            i += 1
                            j -= 1
                        else:
                            groups.append([rest[j]])
                            j -= 1
                    groups.sort(key=lambda g: sum(e[2] - e[1] for e in g))
                    groups = [[jbs[0]]] + groups
                else:
                    groups = [[e] for e in jbs]
                jbs = [e for g in groups for e in g]
                cf = {}
                cl = {}
                for jb, lo, hi in jbs:
                    for c in range(ilo // 512, (ihi + 511) // 512):
                        if lo < (c + 1) * 512 and hi > c * 512:
                            if c not in cf:
                                cf[c] = jb
                            cl[c] = jb
                for ent in jbs:
                    for c in range(ilo // 512, (ihi + 511) // 512):
                        if cf.get(c) == ent[0]:
                            ent[1] = min(ent[1], max(c * 512, ilo))
                            ent[2] = max(ent[2], min((c + 1) * 512, ihi))

                if pv is None:
                    pv = pvp.tile([HD + 1, i_tile], F32, tag="pv", name="pv")
                eff_depth = min(pipe_depth, max(3, len(jbs) // 2))
                h2, poff = (h * HD) // 128, (h * HD) % 128
                for gdx, group in enumerate(groups):
                    st = stp.tile([128, i_tile], F32, tag="st", name="st")
                    off = 0
                    members = []
                    for jb, lo, hi in group:
                        kslice = kTt[jb][poff : poff + HD, h2, :]
                        for l in range(lo, hi, 128):
                            p = off + (l - lo)
                            ic = (i0 + l) // 128
                            bt = btype[(jb, ic)]
                            masked = bt != "T"
                            if masked:
                                # additive mask pre-write; QK accumulates on
                                # top with start=False
                                bi = zero_idx if bt == "F" else bt
                                nc.tensor.matmul(
                                    st[:, p : p + 128],
                                    lhsT=identb,
                                    rhs=msb[:, bi * 128 : (bi + 1) * 128],
                                    start=True,
                                    stop=False,
                                )
                            nc.tensor.matmul(
                                st[:, p : p + 128],
                                lhsT=kslice,
                                rhs=qTt[ic][poff : poff + HD, h2, :],
                                start=not masked,
                                stop=True,
                            )
                        members.append((jb, lo, hi, off))
                        off += hi - lo
                    service()
                    pops = 0
                    while len(pending) >= eff_depth and pops < 2:
                        run_pv(pending.pop(0))
                        pops += 1
                    if filler_q and tick[0] % 2 == 0:
                        filler_q.pop(0)()
                    pt = ptp.tile([128, i_tile], BF16, tag="pt", name="pt")
                    nc.scalar.activation(
                        out=pt[:, 0:off],
                        in_=st[:, 0:off],
                        func=mybir.ActivationFunctionType.Exp,
                        scale=float(scale),
                    )
                    for mdx, (jb, lo, hi, po) in enumerate(members):
                        last = (
                            (it, h)
                            if close
                            and gdx == len(groups) - 1
                            and mdx == len(members) - 1
                            else None
                        )
                        pending.append((pt, pv, h, (jb, lo, hi, po), cf, cl, i0, last))
                    tick[0] += 1
                return pv

            # ---- slot schedule with late-staging injections ----
            n_slots = n_it * heads
            for si in range(n_slots):
                it, h = si // heads, si % heads
                if it > 0:
                    # all late staging must be emitted before i-tile 1 reads it
                    while filler_q:
                        filler_q.pop(0)()
                depth = PIPE
                if si == n_slots - 1:
                    depth = 3
                    eager_state[((n_it - 1) * i_tile, heads - 1)] = {"outT": None}
                if si == 0 and i_tile > 512:
                    # sub-range the first slot so the first exps only need
                    # the first Q/K eighth (staging DMA latency); forward jb
                    # order matches staging arrival order
                    pv0 = emit_slot(it, h, depth, 0, 256, close=False, reorder=False)
                    emit_slot(it, h, depth, 256, 512, pv=pv0, close=False, reorder=False)
                    emit_slot(it, h, depth, 512, i_tile, pv=pv0, close=True, reorder=False)
                else:
                    # final slot: forward order so chunk 0 closes early for
                    # the eager tail norm
                    emit_slot(it, h, depth, reorder=(si != n_slots - 1))
                if it == 0 and n_it > 1:
                    if h == 0:
                        late["k8"] = load(ks, half, half // 2, "natL")
                        late["k12"] = load(ks, half + half // 2, half // 2, "natL")

                        def mk_qk(tiles, nat, a, t):
                            return lambda: stage_qk(tiles, nat, a, t)

                        for t in range(half // 2):
                            filler_q.append(mk_qk(kTt, late["k8"], t, half + t))
                    elif h == 1:
                        late["q8"] = load(qs, half, half // 2, "natL")
                        late["q12"] = load(qs, half + half // 2, half // 2, "natL")

                        def mk_qk(tiles, nat, a, t):
                            return lambda: stage_qk(tiles, nat, a, t)

                        for t in range(half // 2):
                            filler_q.append(
                                mk_qk(kTt, late["k12"], t, half + half // 2 + t)
                            )
                        for t in range(half // 2):
                            filler_q.append(mk_qk(qTt, late["q8"], t, half + t))
                        for t in range(half // 2):
                            filler_q.append(
                                mk_qk(qTt, late["q12"], t, half + half // 2 + t)
                            )
                    elif h == 2:
                        late["v8"] = load(vs, half, half // 2, "natL")
                        late["v12"] = load(vs, half + half // 2, half // 2, "natL")
                    elif h == 6:
                        for t in range(half // 2):
                            stage_v(late["v8"], t, half + t)
                    elif h == 7:
                        for t in range(half // 2):
                            stage_v(late["v12"], t, half + half // 2 + t)

            # ---- drain ----
            while pending:
                run_pv(pending.pop(0))
                tick[0] += 1
                service()
            tick[0] += 10**6
            service()

        if repeat == 1:
            body()
        else:
            with tc.For_i(0, repeat, 1):
                body()

    nc.compile()
    return nc


def _get_program(mask: np.ndarray):
    _lazy_imports()
    key = hash(mask.tobytes())
    if key not in _cache:
        btype, blocks, zero_idx = classify_mask(mask)
        mblk = np.stack(blocks).astype(ml_dtypes.bfloat16)
        nc = build_attn(
            n_cores=N_CORES,
            seq=N,
            heads=HEADS_PER_CORE,
            btype=btype,
            n_blocks=len(blocks),
            zero_idx=zero_idx,
            i_tile=1024,
        )
        _cache[key] = (nc, mblk)
    return _cache[key]


def make_in_maps(q, k, v, mblk):
    in_maps = []
    for c in range(N_CORES):
        b, dg = c // 2, D_CORE * (c % 2)
        in_maps.append(
            {
                "qs": np.ascontiguousarray(q[b][:, dg : dg + D_CORE]),
                "ks": np.ascontiguousarray(k[b][:, dg : dg + D_CORE]),
                "vs": np.ascontiguousarray(v[b][:, dg : dg + D_CORE]),
                "mblk": mblk,
            }
        )
    return in_maps


def gather_out(results):
    y = np.empty((B, N, H * HD), np.float32)
    for c in range(N_CORES):
        b, dg = c // 2, D_CORE * (c % 2)
        y[b][:, dg : dg + D_CORE] = results[c]["ys"]
    return y


def kernel(q, k, v, attn_mask):
    q = np.asarray(q, np.float32)
    k = np.asarray(k, np.float32)
    v = np.asarray(v, np.float32)
    mask = np.asarray(attn_mask, bool)
    nc, mblk = _get_program(mask)
    res = bass_utils.run_bass_kernel_spmd(
        nc, make_in_maps(q, k, v, mblk), core_ids=list(range(N_CORES))
    )
    return gather_out(res.results)
